# revision 32
# baseline (speedup 1.0000x reference)
"""Trainium2 Bass kernel for nn_Attention_59528246723073.

Reference (per batch b, channel c; x[b,c] is [S=256, T=64]):
    fs = tanh(x @ Wspect[c])            # [S]   (contract T)
    ft = tanh(x.T @ Wtemp[c])           # [T]   (contract S)
    a  = softmax_S(fs) * 100
    g  = softmax_T(ft)
    out[b,c,s,t] = x[b,c,s,t] * a[s] * g[t]

Distribution: data-parallel over batch B=32 -> 4 per core on 8 cores.

Per-core layout: [128 part = channels, S*T free] fp16 tiles.  VectorE
(the bottleneck: ~190us busy of ~205us wall) runs everything in fp16
2x_1p mode; all HBM I/O is fp16 (x pre-cast on the host -- bit-identical
to the old on-load cast -- and out stored fp16/upcast on the host),
halving HBM traffic to ~34 MB/core, far below the ~358 GB/s/core limit.
Loads ride the SP HWDGE ring, stores the ACT HWDGE ring (two
independent FIFOs, no SWDGE desc-gen serialization).

"Shared-q" structure (5 DVE passes/batch instead of the naive 6):
  W[c,s,t] = fp16(Wtemp[c,s])*fp16(Wspect[c,t]) is precomputed on the
  HOST (weight preprocessing) along with f32 reciprocals 1/Wtemp,
  1/Wspect.  Per batch ONE mul q = x*W feeds BOTH reductions:
      sum_t q = Wtemp[s]  * fs[s]      (recover fs = . * rwt, f32)
      sum_s q = Wspect[t] * ft[t]      (recover ft = . * rws, f32)
  The weight factor is constant across each reduction axis, so fp16
  rounding errors scale out with it and the recovery is exact to
  fp16-chain accuracy (1.05e-2 measured vs the 2e-2 gate; near-zero
  weights only risk fp16 subnormals in q, negligible after recovery).

Per chunk (s-rows): q-mul -> fold_t level 1 NON-destructively into a
half-size buffer h (so q survives) -> fold chain in h -> per-s sums;
then fold_s destroys q (flat halving) -> per-t sums.  The LAST chunk's
fold_s is deferred until after the fs recovery, so ScalarE's fs
tanh/exp/a2 round trip hides behind ~4.4us of DVE fold_s work.

Schedule notes:
  - Batch 0 uses graduated chunks (32,32,64,128 s-rows; powers of
    two for the halving chains) so compute starts after ~0.5 MB
    lands; W loads in matching pieces on the (otherwise idle at
    start) ACT ring; batch 1 is split in halves (HBM is contended at
    startup while all 8 cores load, and batch 0's compute is short);
    batches 2-3 are a single 256-row chunk (fewer fixed-cost ops)
    with one 4 MB load emitted two batches ahead (3 X2 buffers).
  - The recovery reciprocal loads sit AFTER batch 0's x pieces on the
    SP ring: they're not needed until batch 0's softmax, and putting
    anything ahead of the x pieces delays the first q-mul (a measured
    9us startup stall when DVE reciprocals sat there instead).
  - Softmax skips max-subtraction (tanh logits in [-1,1]); exp's
    fused accum_out provides the sum; a2 paired-duplicate keeps the
    a-mul in 2x mode; finals do a-mul before g-mul (a2 ready first).
  - Finals for batches 0-2 run IN PLACE over X2 (x is dead after
    them) in 2 pieces; the last batch's finals stage through a 4-deep
    oc pool in eighths instead: in-place there made each piece's
    a-mul WAR-wait on the previous piece's store receipt, and small
    pieces shrink the exposed store tail (~2us).
Rejected via HW probes (v1, see kernel_baseline.py): DMA accum_op (NRT
crash), GpSimd tensor_tensor (16x slower), ScalarE strided accum
sweeps, tensor_reduce / scan (1x-mode), fold-op merging (DRAIN scales
with op duration).  Custom DVE specs run at 1x (no fused mul+fold win);
ACT activation accum_out is [P,1]-only (no segmented folds off-DVE);
PE is unusable (per-channel weights on the partition axis everywhere).
"""

import numpy as np

import concourse.bass as bass
import concourse.tile as tile
from concourse import bacc, mybir
from concourse.bass_utils import run_bass_kernel_spmd

B, C, S, T = 32, 128, 256, 64
N_CORES = 8
B_LOC = B // N_CORES
F32 = mybir.dt.float32
F16 = mybir.dt.float16

_NC = None


def build_nc():
    nc = bacc.Bacc("TRN2", target_bir_lowering=False, debug=False)
    # x is pre-cast to fp16 on the host (the kernel computes in fp16
    # anyway, so the SBUF contents are bit-identical to the old SWDGE
    # cast-load) and out is stored fp16 and upcast on the host (the
    # final tile is fp16 before the store either way).  This halves
    # HBM traffic (67 -> 34 MB/core), moves DMA far off the critical
    # path, and lets loads/stores ride the two independent HWDGE
    # rings (SP for loads, ACT for stores) with no SWDGE desc-gen.
    x = nc.dram_tensor("x", [B_LOC, C, S, T], F16, kind="ExternalInput")
    # host-precomputed weight products (input-only data, replicated to
    # all cores): W[c,s,t] = fp16(fp16(Wtemp)*fp16(Wspect)) for the
    # shared q-mul, and f32 reciprocals 1/Wtemp, 1/Wspect for the
    # fs/ft recovery.  Removes the device-side W build AND the two DVE
    # reciprocals that used to sit ahead of the first q-mul in DVE
    # program order (measured 9us of startup stall).
    win = nc.dram_tensor("wouter", [C, S * T], F16, kind="ExternalInput")
    rwsin = nc.dram_tensor("rws", [C, T], F32, kind="ExternalInput")
    rwtin = nc.dram_tensor("rwt", [C, S], F32, kind="ExternalInput")
    out = nc.dram_tensor("out", [B_LOC, C, S, T], F16, kind="ExternalOutput")

    AF = mybir.ActivationFunctionType
    OP = mybir.AluOpType

    with tile.TileContext(nc) as tc:
        with (
            tc.tile_pool(name="consts", bufs=1) as cpool,
            tc.tile_pool(name="x2", bufs=3) as x2pool,
            tc.tile_pool(name="q", bufs=1) as qpool,
            tc.tile_pool(name="h", bufs=1) as hpool,
            tc.tile_pool(name="ocp", bufs=3) as ocpool,
            tc.tile_pool(name="small", bufs=2) as spool,
        ):
            # --- constants ---
            # W pieces on the ACT queue (idle until the first store) in
            # batch-0-chunk-sized pieces so the first q-mul only waits
            # for ~0.25 MB on each ring; the recovery reciprocals ride
            # the SP queue AFTER batch 0's x pieces (not needed until
            # batch 0's softmax).
            Wt_ = cpool.tile([C, S * T], F16)
            W3 = Wt_.rearrange("p (s t) -> p s t", t=T)
            for q0, q1 in ((0, 32), (32, 64), (64, 128), (128, 256)):
                nc.scalar.dma_start(
                    Wt_[:, q0 * T : q1 * T], win[:, q0 * T : q1 * T]
                )
            rws = cpool.tile([C, T], F32)
            rwt = cpool.tile([C, S], F32)

            def emit_loads(b, X2, loads, start=0):
                with nc.named_scope("load"):
                    q0 = start
                    for ln in loads:
                        sq = slice(q0, q0 + ln)
                        nc.sync.dma_start(
                            X2[:, sq.start * T : sq.stop * T], x[b, :, sq, :]
                        )
                        q0 += ln

            X2_tiles = {}
            X2_tiles[0] = x2pool.tile([C, S * T], F16, tag="X2", name="X2b0")
            emit_loads(0, X2_tiles[0], (32, 32, 64, 64, 64))
            nc.sync.dma_start(rws[:], rwsin[:])
            nc.sync.dma_start(rwt[:], rwtin[:])
            if B_LOC > 1:
                # batch 1 is split in halves ACROSS THE TWO HWDGE
                # rings: at startup all 8 cores load concurrently
                # (HBM-contended) and batch 0's compute is short, so
                # b1 trailing b0's 4 MB on the SP ring left a 0.8-4us
                # DVE gap at the b0->b1 transition.  Half on SP (after
                # b0), half on ACT (after the W pieces) lands both by
                # ~21us.
                X2_tiles[1] = x2pool.tile([C, S * T], F16, tag="X2", name="X2b1")
                with nc.named_scope("load"):
                    nc.sync.dma_start(
                        X2_tiles[1][:, 0 : 128 * T], x[1, :, 0:128, :]
                    )
                    nc.scalar.dma_start(
                        X2_tiles[1][:, 128 * T : 256 * T], x[1, :, 128:256, :]
                    )

            for b in range(B_LOC):
                X2 = X2_tiles.pop(b)
                X23 = X2.rearrange("p (s t) -> p s t", t=T)
                # wtfs becomes fs (and wsft becomes ft) after the
                # in-place reciprocal recovery.
                fs = spool.tile([C, S], F32, tag="fs")
                ft = spool.tile([C, T], F32, tag="ft")

                if b == 0:
                    chunks = (32, 32, 64, 128)
                elif b == 1:
                    chunks = (128, 128)
                else:
                    chunks = (256,)

                def fold_s_levels(xcf, w, first):
                    # halving chain from width w down to 2T, then the
                    # combine into the ft accumulator
                    while w >= 2 * T:
                        nc.vector.tensor_tensor(
                            xcf[:, 0:w], xcf[:, 0:w], xcf[:, w : 2 * w],
                            op=OP.add,
                        )
                        w //= 2
                    if first:
                        nc.vector.tensor_tensor(
                            ft[:], xcf[:, 0:T], xcf[:, T : 2 * T], op=OP.add
                        )
                    else:
                        nc.vector.tensor_tensor(
                            xcf[:, 0:T], xcf[:, 0:T], xcf[:, T : 2 * T],
                            op=OP.add,
                        )
                        nc.vector.tensor_tensor(
                            ft[:], ft[:], xcf[:, 0:T], op=OP.add
                        )

                def fold_s(xcf, sc, first):
                    # sum over s of the (destroyed) q chunk -> ft accum
                    fold_s_levels(xcf, sc * T // 2, first)

                s0 = 0
                last_fold_s = None
                for k, sc in enumerate(chunks):
                    sl = slice(s0, s0 + sc)
                    qt = qpool.tile([C, sc * T], F16, tag="q")
                    q3 = qt.rearrange("p (s t) -> p s t", t=T)
                    with nc.named_scope("qmul"):
                        nc.vector.tensor_tensor(
                            q3, X23[:, sl, :], W3[:, sl, :], op=OP.mult
                        )
                    # fs path: fold over t.  Level 1 writes the
                    # half-size h buffer so q survives for fold_s.
                    with nc.named_scope("fs"):
                        ht = hpool.tile([C, sc * (T // 2)], F16, tag="h")
                        h3 = ht.rearrange("p (s t) -> p s t", t=T // 2)
                        nc.vector.tensor_tensor(
                            h3, q3[:, :, 0 : T // 2],
                            q3[:, :, T // 2 : T], op=OP.add,
                        )
                        w = T // 4
                        while w >= 2:
                            nc.vector.tensor_tensor(
                                h3[:, :, 0:w], h3[:, :, 0:w],
                                h3[:, :, w : 2 * w], op=OP.add,
                            )
                            w //= 2
                        nc.vector.tensor_tensor(
                            fs[:, sl], h3[:, :, 0:1], h3[:, :, 1:2], op=OP.add
                        )
                    # ft path: defer the LAST chunk's fold_s until after
                    # the fs recovery, so ScalarE's fs softmax chain
                    # overlaps ~4us of DVE fold_s work.
                    if k == len(chunks) - 1:
                        last_fold_s = (qt, sc, k == 0)
                    else:
                        with nc.named_scope("ft"):
                            fold_s(qt, sc, k == 0)
                    s0 += sc

                # emit loads two batches ahead, BEFORE this batch's
                # stores enter the Pool queue (in-order SWDGE desc-gen).
                if b + 2 < B_LOC:
                    X2_tiles[b + 2] = x2pool.tile(
                        [C, S * T], F16, tag="X2", name=f"X2b{b+2}"
                    )
                    emit_loads(b + 2, X2_tiles[b + 2], (256,))

                with nc.named_scope("softmax"):
                    # recover fs = wtfs / wt (f32; exact cancellation of
                    # the wt factor), then the usual softmax-sans-max.
                    nc.vector.tensor_tensor(fs[:], fs[:], rwt[:], op=OP.mult)
                    ssum = spool.tile([C, 1], F32, tag="ssum")
                    rec = spool.tile([C, 1], F32, tag="rec")
                    nc.scalar.activation(fs[:], fs[:], AF.Tanh)
                    nc.scalar.activation(
                        fs[:], fs[:], AF.Exp, accum_out=ssum[:, 0:1]
                    )

                # the deferred fold_s brackets the fs reciprocal: level
                # 1 (~half the chain's time) covers ScalarE's tanh/exp,
                # the reciprocal lands mid-chain with its input ready,
                # and ScalarE's a2 build then overlaps the remaining
                # levels so the first a-mul never waits on a2.
                qt_l, sc_l, first_l = last_fold_s
                with nc.named_scope("ft"):
                    wl = sc_l * T // 2
                    nc.vector.tensor_tensor(
                        qt_l[:, 0:wl], qt_l[:, 0:wl],
                        qt_l[:, wl : 2 * wl], op=OP.add,
                    )
                with nc.named_scope("softmax"):
                    nc.vector.reciprocal(rec[:], ssum[:])
                    a2 = spool.tile([C, 2 * S], F16, tag="a2")
                    nc.scalar.activation(
                        a2.rearrange("p (s j) -> p s j", j=2),
                        fs[:].unsqueeze(2).to_broadcast((C, S, 2)),
                        AF.Copy,
                        scale=rec[:, 0:1],
                    )
                with nc.named_scope("ft"):
                    fold_s_levels(qt_l, sc_l * T // 4, first_l)
                with nc.named_scope("softmax"):
                    nc.vector.tensor_tensor(ft[:], ft[:], rws[:], op=OP.mult)
                    ssum2 = spool.tile([C, 1], F32, tag="ssum2")
                    rec2 = spool.tile([C, 1], F32, tag="rec2")
                    nc.scalar.activation(ft[:], ft[:], AF.Tanh)
                    nc.scalar.activation(
                        ft[:], ft[:], AF.Exp, accum_out=ssum2[:, 0:1]
                    )

                # final multiplies IN PLACE over X2 (x is dead after
                # them; no oc staging pool, no store-gated buffer
                # waits) + store; small pieces on the last batch so the
                # tail out-DMA exposure is short.  The FIRST piece's
                # a-mul is emitted between the ft exp and its
                # reciprocal (it only needs a2), hiding that ScalarE
                # round trip behind DVE work too.
                if b == B_LOC - 1:
                    # first piece 64 rows: its a-mul covers the ft
                    # softmax round trip; small pieces after so the
                    # exposed store tail stays ~2us.
                    pieces = [(0, 64)] + [
                        (64 + k * 32, 32) for k in range(5)
                    ] + [(224, 16), (240, 16)]
                else:
                    # one piece: 2 fewer ops/batch, and the 8.7us a-mul
                    # fully hides ScalarE's ft tanh/exp before recip2.
                    pieces = [(0, 256)]

                oc_tiles = {}

                def emit_amul(p0, pn):
                    sl = slice(p0, p0 + pn)
                    with nc.named_scope("final"):
                        xf = X2[:, sl.start * T : sl.stop * T]
                        if b == B_LOC - 1:
                            # last batch: stage through oc so the next
                            # piece's a-mul doesn't WAR-wait on this
                            # piece's store receipt (X2 is written in
                            # place otherwise).
                            oc = ocpool.tile([C, pn * T], F16, tag="oc")
                        else:
                            oc = xf
                        oc_tiles[p0] = oc
                        oP = oc.rearrange(
                            "p (s pr j) -> p s pr j", pr=T // 2, j=2
                        )
                        xP = xf.rearrange(
                            "p (s pr j) -> p s pr j", pr=T // 2, j=2
                        )
                        aP = (
                            a2[:, 2 * p0 : 2 * (p0 + pn)]
                            .rearrange("p (s j) -> p s j", j=2)
                            .unsqueeze(2)
                            .to_broadcast((C, pn, T // 2, 2))
                        )
                        nc.vector.tensor_tensor(oP, xP, aP, op=OP.mult)

                def emit_gmul_store(p0, pn):
                    sl = slice(p0, p0 + pn)
                    with nc.named_scope("final"):
                        oc = oc_tiles.pop(p0)
                        o3 = oc.rearrange("p (s t) -> p s t", t=T)
                        g_bcq = g16.unsqueeze(1).to_broadcast((C, pn, T))
                        nc.vector.tensor_tensor(o3, o3, g_bcq, op=OP.mult)
                        nc.scalar.dma_start(out[b, :, sl, :], oc[:])

                emit_amul(*pieces[0])
                with nc.named_scope("softmax"):
                    nc.vector.reciprocal(rec2[:], ssum2[:])
                    g16 = spool.tile([C, T], F16, tag="g16")
                    nc.vector.tensor_scalar(
                        out=g16[:], in0=ft[:], scalar1=rec2[:, 0:1],
                        scalar2=100.0, op0=OP.mult, op1=OP.mult,
                    )
                emit_gmul_store(*pieces[0])
                for p0, pn in pieces[1:]:
                    emit_amul(p0, pn)
                    emit_gmul_store(p0, pn)

    nc.compile()
    return nc


def get_nc():
    global _NC
    if _NC is None:
        _NC = build_nc()
    return _NC


def shard_inputs(x, Wspect, Wtemp):
    ws = Wspect.reshape(C, T).astype(np.float32)
    wt = Wtemp.reshape(C, S).astype(np.float32)
    # host-side fp16 pre-cast: bit-identical to the kernel's old
    # on-load SWDGE cast, at half the HBM load traffic.
    x = np.ascontiguousarray(x.astype(np.float16))
    wouter = np.ascontiguousarray(
        (wt.astype(np.float16)[:, :, None] * ws.astype(np.float16)[:, None, :])
        .astype(np.float16)
        .reshape(C, S * T)
    )
    rws = np.ascontiguousarray((1.0 / ws).astype(np.float32))
    rwt = np.ascontiguousarray((1.0 / wt).astype(np.float32))
    return [
        {
            "x": x[i * B_LOC : (i + 1) * B_LOC],
            "wouter": wouter,
            "rws": rws,
            "rwt": rwt,
        }
        for i in range(N_CORES)
    ]


def unshard(results):
    return np.concatenate([r["out"] for r in results], axis=0).astype(
        np.float32
    )


def kernel(x, Wspect, Wtemp):
    nc = get_nc()
    in_maps = shard_inputs(x, Wspect, Wtemp)
    res = run_bass_kernel_spmd(nc, in_maps, core_ids=list(range(N_CORES)))
    return unshard(res.results)


# revision 33
# speedup vs baseline: 1.0096x; 1.0096x over previous
"""Trainium2 Bass kernel for nn_Attention_59528246723073.

Reference (per batch b, channel c; x[b,c] is [S=256, T=64]):
    fs = tanh(x @ Wspect[c])            # [S]   (contract T)
    ft = tanh(x.T @ Wtemp[c])           # [T]   (contract S)
    a  = softmax_S(fs) * 100
    g  = softmax_T(ft)
    out[b,c,s,t] = x[b,c,s,t] * a[s] * g[t]

Distribution: data-parallel over batch B=32 -> 4 per core on 8 cores.

Per-core layout: [128 part = channels, S*T free] fp16 tiles.  VectorE
(the bottleneck: ~190us busy of ~205us wall) runs everything in fp16
2x_1p mode; all HBM I/O is fp16 (x pre-cast on the host -- bit-identical
to the old on-load cast -- and out stored fp16/upcast on the host),
halving HBM traffic to ~34 MB/core, far below the ~358 GB/s/core limit.
Loads ride the SP HWDGE ring, stores the ACT HWDGE ring (two
independent FIFOs, no SWDGE desc-gen serialization).

"Shared-q" structure (5 DVE passes/batch instead of the naive 6):
  W[c,s,t] = fp16(Wtemp[c,s])*fp16(Wspect[c,t]) is precomputed on the
  HOST (weight preprocessing) along with f32 reciprocals 1/Wtemp,
  1/Wspect.  Per batch ONE mul q = x*W feeds BOTH reductions:
      sum_t q = Wtemp[s]  * fs[s]      (recover fs = . * rwt, f32)
      sum_s q = Wspect[t] * ft[t]      (recover ft = . * rws, f32)
  The weight factor is constant across each reduction axis, so fp16
  rounding errors scale out with it and the recovery is exact to
  fp16-chain accuracy (1.05e-2 measured vs the 2e-2 gate; near-zero
  weights only risk fp16 subnormals in q, negligible after recovery).

Per chunk (s-rows): q-mul -> fold_t level 1 NON-destructively into a
half-size buffer h (so q survives) -> fold chain in h -> per-s sums;
then fold_s destroys q (flat halving) -> per-t sums.  The LAST chunk's
fold_s is deferred until after the fs recovery, so ScalarE's fs
tanh/exp/a2 round trip hides behind ~4.4us of DVE fold_s work.

Schedule notes:
  - Batch 0 uses graduated chunks (32,32,64,128 s-rows; powers of
    two for the halving chains) so compute starts after ~0.5 MB
    lands; W loads in matching pieces on the (otherwise idle at
    start) ACT ring; batch 1 is split in halves (HBM is contended at
    startup while all 8 cores load, and batch 0's compute is short);
    batches 2-3 are a single 256-row chunk (fewer fixed-cost ops)
    with one 4 MB load emitted two batches ahead (3 X2 buffers).
  - The recovery reciprocal loads sit AFTER batch 0's x pieces on the
    SP ring: they're not needed until batch 0's softmax, and putting
    anything ahead of the x pieces delays the first q-mul (a measured
    9us startup stall when DVE reciprocals sat there instead).
  - Softmax skips max-subtraction (tanh logits in [-1,1]); exp's
    fused accum_out provides the sum; a2 paired-duplicate keeps the
    a-mul in 2x mode; finals do a-mul before g-mul (a2 ready first).
  - Finals for batches 0-2 run IN PLACE over X2 (x is dead after
    them) in 2 pieces; the last batch's finals stage through a 4-deep
    oc pool in eighths instead: in-place there made each piece's
    a-mul WAR-wait on the previous piece's store receipt, and small
    pieces shrink the exposed store tail (~2us).
Rejected via HW probes (v1, see kernel_baseline.py): DMA accum_op (NRT
crash), GpSimd tensor_tensor (16x slower), ScalarE strided accum
sweeps, tensor_reduce / scan (1x-mode), fold-op merging (DRAIN scales
with op duration).  Custom DVE specs run at 1x (no fused mul+fold win);
ACT activation accum_out is [P,1]-only (no segmented folds off-DVE);
PE is unusable (per-channel weights on the partition axis everywhere).
"""

import numpy as np

import concourse.bass as bass
import concourse.tile as tile
from concourse import bacc, mybir
from concourse.bass_utils import run_bass_kernel_spmd

B, C, S, T = 32, 128, 256, 64
N_CORES = 8
B_LOC = B // N_CORES
F32 = mybir.dt.float32
F16 = mybir.dt.float16

_NC = None


def build_nc():
    nc = bacc.Bacc("TRN2", target_bir_lowering=False, debug=False)
    # x is pre-cast to fp16 on the host (the kernel computes in fp16
    # anyway, so the SBUF contents are bit-identical to the old SWDGE
    # cast-load) and out is stored fp16 and upcast on the host (the
    # final tile is fp16 before the store either way).  This halves
    # HBM traffic (67 -> 34 MB/core), moves DMA far off the critical
    # path, and lets loads/stores ride the two independent HWDGE
    # rings (SP for loads, ACT for stores) with no SWDGE desc-gen.
    x = nc.dram_tensor("x", [B_LOC, C, S, T], F16, kind="ExternalInput")
    # host-precomputed weight products (input-only data, replicated to
    # all cores): W[c,s,t] = fp16(fp16(Wtemp)*fp16(Wspect)) for the
    # shared q-mul, and f32 reciprocals 1/Wtemp, 1/Wspect for the
    # fs/ft recovery.  Removes the device-side W build AND the two DVE
    # reciprocals that used to sit ahead of the first q-mul in DVE
    # program order (measured 9us of startup stall).
    win = nc.dram_tensor("wouter", [C, S * T], F16, kind="ExternalInput")
    rwsin = nc.dram_tensor("rws", [C, T], F32, kind="ExternalInput")
    rwtin = nc.dram_tensor("rwt", [C, S], F32, kind="ExternalInput")
    out = nc.dram_tensor("out", [B_LOC, C, S, T], F16, kind="ExternalOutput")

    AF = mybir.ActivationFunctionType
    OP = mybir.AluOpType

    with tile.TileContext(nc) as tc:
        with (
            tc.tile_pool(name="consts", bufs=1) as cpool,
            tc.tile_pool(name="x2", bufs=3) as x2pool,
            tc.tile_pool(name="q", bufs=1) as qpool,
            tc.tile_pool(name="h", bufs=1) as hpool,
            tc.tile_pool(name="ocp", bufs=3) as ocpool,
            tc.tile_pool(name="small", bufs=2) as spool,
        ):
            # --- constants ---
            # W pieces on the ACT queue (idle until the first store) in
            # batch-0-chunk-sized pieces so the first q-mul only waits
            # for ~0.25 MB on each ring; the recovery reciprocals ride
            # the SP queue AFTER batch 0's x pieces (not needed until
            # batch 0's softmax).
            Wt_ = cpool.tile([C, S * T], F16)
            W3 = Wt_.rearrange("p (s t) -> p s t", t=T)
            for q0, q1 in ((0, 32), (32, 64), (64, 128), (128, 256)):
                nc.scalar.dma_start(
                    Wt_[:, q0 * T : q1 * T], win[:, q0 * T : q1 * T]
                )
            rws = cpool.tile([C, T], F32)
            rwt = cpool.tile([C, S], F32)

            def emit_loads(b, X2, loads, start=0):
                with nc.named_scope("load"):
                    q0 = start
                    for ln in loads:
                        sq = slice(q0, q0 + ln)
                        nc.sync.dma_start(
                            X2[:, sq.start * T : sq.stop * T], x[b, :, sq, :]
                        )
                        q0 += ln

            X2_tiles = {}
            X2_tiles[0] = x2pool.tile([C, S * T], F16, tag="X2", name="X2b0")
            emit_loads(0, X2_tiles[0], (32, 32, 64, 64, 64))
            nc.sync.dma_start(rws[:], rwsin[:])
            nc.sync.dma_start(rwt[:], rwtin[:])
            if B_LOC > 1:
                # batch 1 is split in halves ACROSS THE TWO HWDGE
                # rings: at startup all 8 cores load concurrently
                # (HBM-contended) and batch 0's compute is short, so
                # b1 trailing b0's 4 MB on the SP ring left a 0.8-4us
                # DVE gap at the b0->b1 transition.  Half on SP (after
                # b0), half on ACT (after the W pieces) lands both by
                # ~21us.
                X2_tiles[1] = x2pool.tile([C, S * T], F16, tag="X2", name="X2b1")
                with nc.named_scope("load"):
                    nc.sync.dma_start(
                        X2_tiles[1][:, 0 : 128 * T], x[1, :, 0:128, :]
                    )
                    nc.scalar.dma_start(
                        X2_tiles[1][:, 128 * T : 256 * T], x[1, :, 128:256, :]
                    )

            for b in range(B_LOC):
                X2 = X2_tiles.pop(b)
                X23 = X2.rearrange("p (s t) -> p s t", t=T)
                # wtfs becomes fs (and wsft becomes ft) after the
                # in-place reciprocal recovery.
                fs = spool.tile([C, S], F32, tag="fs")
                ft = spool.tile([C, T], F32, tag="ft")

                if b == 0:
                    chunks = (32, 32, 64, 128)
                elif b == 1:
                    chunks = (128, 128)
                else:
                    chunks = (256,)

                def fold_s_levels(xcf, w, first):
                    # halving chain from width w down to 2T, then the
                    # combine into the ft accumulator
                    while w >= 2 * T:
                        nc.vector.tensor_tensor(
                            xcf[:, 0:w], xcf[:, 0:w], xcf[:, w : 2 * w],
                            op=OP.add,
                        )
                        w //= 2
                    if first:
                        nc.vector.tensor_tensor(
                            ft[:], xcf[:, 0:T], xcf[:, T : 2 * T], op=OP.add
                        )
                    else:
                        nc.vector.tensor_tensor(
                            xcf[:, 0:T], xcf[:, 0:T], xcf[:, T : 2 * T],
                            op=OP.add,
                        )
                        nc.vector.tensor_tensor(
                            ft[:], ft[:], xcf[:, 0:T], op=OP.add
                        )

                def fold_s(xcf, sc, first):
                    # sum over s of the (destroyed) q chunk -> ft accum
                    fold_s_levels(xcf, sc * T // 2, first)

                s0 = 0
                last_fold_s = None
                for k, sc in enumerate(chunks):
                    sl = slice(s0, s0 + sc)
                    qt = qpool.tile([C, sc * T], F16, tag="q")
                    q3 = qt.rearrange("p (s t) -> p s t", t=T)
                    with nc.named_scope("qmul"):
                        nc.vector.tensor_tensor(
                            q3, X23[:, sl, :], W3[:, sl, :], op=OP.mult
                        )
                    # fs path: fold over t.  Level 1 writes the
                    # half-size h buffer so q survives for fold_s.
                    with nc.named_scope("fs"):
                        ht = hpool.tile([C, sc * (T // 2)], F16, tag="h")
                        h3 = ht.rearrange("p (s t) -> p s t", t=T // 2)
                        nc.vector.tensor_tensor(
                            h3, q3[:, :, 0 : T // 2],
                            q3[:, :, T // 2 : T], op=OP.add,
                        )
                        w = T // 4
                        while w >= 2:
                            nc.vector.tensor_tensor(
                                h3[:, :, 0:w], h3[:, :, 0:w],
                                h3[:, :, w : 2 * w], op=OP.add,
                            )
                            w //= 2
                        nc.vector.tensor_tensor(
                            fs[:, sl], h3[:, :, 0:1], h3[:, :, 1:2], op=OP.add
                        )
                    # ft path: defer the LAST chunk's fold_s until after
                    # the fs recovery, so ScalarE's fs softmax chain
                    # overlaps ~4us of DVE fold_s work.
                    if k == len(chunks) - 1:
                        last_fold_s = (qt, sc, k == 0)
                    else:
                        with nc.named_scope("ft"):
                            fold_s(qt, sc, k == 0)
                    s0 += sc

                # emit loads two batches ahead, BEFORE this batch's
                # stores enter the Pool queue (in-order SWDGE desc-gen).
                if b + 2 < B_LOC:
                    X2_tiles[b + 2] = x2pool.tile(
                        [C, S * T], F16, tag="X2", name=f"X2b{b+2}"
                    )
                    emit_loads(b + 2, X2_tiles[b + 2], (256,))

                with nc.named_scope("softmax"):
                    # recover fs = wtfs / wt (f32; exact cancellation of
                    # the wt factor), then the usual softmax-sans-max.
                    nc.vector.tensor_tensor(fs[:], fs[:], rwt[:], op=OP.mult)
                    ssum = spool.tile([C, 1], F32, tag="ssum")
                    rec = spool.tile([C, 1], F32, tag="rec")
                    nc.scalar.activation(fs[:], fs[:], AF.Tanh)
                    nc.scalar.activation(
                        fs[:], fs[:], AF.Exp, accum_out=ssum[:, 0:1]
                    )

                # the deferred fold_s brackets the fs reciprocal: level
                # 1 (~half the chain's time) covers ScalarE's tanh/exp,
                # the reciprocal lands mid-chain with its input ready,
                # and ScalarE's a2 build then overlaps the remaining
                # levels so the first a-mul never waits on a2.
                qt_l, sc_l, first_l = last_fold_s
                with nc.named_scope("ft"):
                    wl = sc_l * T // 2
                    nc.vector.tensor_tensor(
                        qt_l[:, 0:wl], qt_l[:, 0:wl],
                        qt_l[:, wl : 2 * wl], op=OP.add,
                    )
                with nc.named_scope("softmax"):
                    nc.vector.reciprocal(rec[:], ssum[:])
                    a2 = spool.tile([C, 2 * S], F16, tag="a2")
                    nc.scalar.activation(
                        a2.rearrange("p (s j) -> p s j", j=2),
                        fs[:].unsqueeze(2).to_broadcast((C, S, 2)),
                        AF.Copy,
                        scale=rec[:, 0:1],
                    )
                with nc.named_scope("ft"):
                    fold_s_levels(qt_l, sc_l * T // 4, first_l)
                with nc.named_scope("softmax"):
                    nc.vector.tensor_tensor(ft[:], ft[:], rws[:], op=OP.mult)
                    ssum2 = spool.tile([C, 1], F32, tag="ssum2")
                    rec2 = spool.tile([C, 1], F32, tag="rec2")
                    nc.scalar.activation(ft[:], ft[:], AF.Tanh)
                    nc.scalar.activation(
                        ft[:], ft[:], AF.Exp, accum_out=ssum2[:, 0:1]
                    )

                # final multiplies IN PLACE over X2 (x is dead after
                # them; no oc staging pool, no store-gated buffer
                # waits) + store; small pieces on the last batch so the
                # tail out-DMA exposure is short.  The FIRST piece's
                # a-mul is emitted between the ft exp and its
                # reciprocal (it only needs a2), hiding that ScalarE
                # round trip behind DVE work too.
                if b == B_LOC - 1:
                    # first piece 64 rows: its a-mul covers the ft
                    # softmax round trip; small pieces after so the
                    # exposed store tail stays ~2us.
                    pieces = [
                        (0, 64), (64, 64), (128, 64),
                        (192, 32), (224, 16), (240, 16),
                    ]
                else:
                    # one piece: 2 fewer ops/batch, and the 8.7us a-mul
                    # fully hides ScalarE's ft tanh/exp before recip2.
                    pieces = [(0, 256)]

                oc_tiles = {}

                def emit_amul(p0, pn):
                    sl = slice(p0, p0 + pn)
                    with nc.named_scope("final"):
                        xf = X2[:, sl.start * T : sl.stop * T]
                        if b == B_LOC - 1:
                            # last batch: stage through oc so the next
                            # piece's a-mul doesn't WAR-wait on this
                            # piece's store receipt (X2 is written in
                            # place otherwise).
                            oc = ocpool.tile([C, pn * T], F16, tag="oc")
                        else:
                            oc = xf
                        oc_tiles[p0] = oc
                        oP = oc.rearrange(
                            "p (s pr j) -> p s pr j", pr=T // 2, j=2
                        )
                        xP = xf.rearrange(
                            "p (s pr j) -> p s pr j", pr=T // 2, j=2
                        )
                        aP = (
                            a2[:, 2 * p0 : 2 * (p0 + pn)]
                            .rearrange("p (s j) -> p s j", j=2)
                            .unsqueeze(2)
                            .to_broadcast((C, pn, T // 2, 2))
                        )
                        nc.vector.tensor_tensor(oP, xP, aP, op=OP.mult)

                def emit_gmul_store(p0, pn):
                    sl = slice(p0, p0 + pn)
                    with nc.named_scope("final"):
                        oc = oc_tiles.pop(p0)
                        o3 = oc.rearrange("p (s t) -> p s t", t=T)
                        g_bcq = g16.unsqueeze(1).to_broadcast((C, pn, T))
                        nc.vector.tensor_tensor(o3, o3, g_bcq, op=OP.mult)
                        nc.scalar.dma_start(out[b, :, sl, :], oc[:])

                emit_amul(*pieces[0])
                with nc.named_scope("softmax"):
                    nc.vector.reciprocal(rec2[:], ssum2[:])
                    g16 = spool.tile([C, T], F16, tag="g16")
                    nc.vector.tensor_scalar(
                        out=g16[:], in0=ft[:], scalar1=rec2[:, 0:1],
                        scalar2=100.0, op0=OP.mult, op1=OP.mult,
                    )
                emit_gmul_store(*pieces[0])
                for p0, pn in pieces[1:]:
                    emit_amul(p0, pn)
                    emit_gmul_store(p0, pn)

    nc.compile()
    return nc


def get_nc():
    global _NC
    if _NC is None:
        _NC = build_nc()
    return _NC


def shard_inputs(x, Wspect, Wtemp):
    ws = Wspect.reshape(C, T).astype(np.float32)
    wt = Wtemp.reshape(C, S).astype(np.float32)
    # host-side fp16 pre-cast: bit-identical to the kernel's old
    # on-load SWDGE cast, at half the HBM load traffic.
    x = np.ascontiguousarray(x.astype(np.float16))
    wouter = np.ascontiguousarray(
        (wt.astype(np.float16)[:, :, None] * ws.astype(np.float16)[:, None, :])
        .astype(np.float16)
        .reshape(C, S * T)
    )
    rws = np.ascontiguousarray((1.0 / ws).astype(np.float32))
    rwt = np.ascontiguousarray((1.0 / wt).astype(np.float32))
    return [
        {
            "x": x[i * B_LOC : (i + 1) * B_LOC],
            "wouter": wouter,
            "rws": rws,
            "rwt": rwt,
        }
        for i in range(N_CORES)
    ]


def unshard(results):
    return np.concatenate([r["out"] for r in results], axis=0).astype(
        np.float32
    )


def kernel(x, Wspect, Wtemp):
    nc = get_nc()
    in_maps = shard_inputs(x, Wspect, Wtemp)
    res = run_bass_kernel_spmd(nc, in_maps, core_ids=list(range(N_CORES)))
    return unshard(res.results)
